# revision 9
# baseline (speedup 1.0000x reference)
"""CRF loss kernel for Trainium2 (8 NeuronCores, data-parallel over batch).

Problem (hardcoded shapes): scores [B=128, T=256, K=64, K=64] f32,
targets [128, 256] int (flattened from_tag*K + to_tag), lengths [128] int.

loss = (sum_b fs[b, END] - gold) / B  where fs is the CRF forward
(log-domain) scan and gold is the gathered gold-path score.

The run is IO-bound (512 MB of scores), so the kernel minimizes bytes
shipped to the device:

  * scores are quantized to fp8 (e4m3) on the host and PACKED: only the
    t < L_b timesteps of each row are sent.  Rows are distributed over
    the 8 cores with a longest-processing-time balance on ceil(L/W).
  * Per core the packed data lives in an "arena" [NW, K, W*K] fp8 of
    W-timestep windows stored kf-major, so a window gather lands
    [kf partitions x W*K free] with W*K-byte contiguous descriptors.
    Window 0 is a shared identity window.
  * Steps t >= L_b multiply the (frozen) state by a constant
    C*Id in exp space (pad blocks: diag S_STAR, off-diag -240 -> exp 0),
    so the final state after all T-1 steps is a_tau * r^(255-tau) with
    r = exp(S_STAR)/C known on the host -> exact correction, and only
    ONE [128, BL] state is read back (vs all T states).
  * Forward scan in the *linear* domain with a constant per-step scale
    1/C: exp(scores) as bf16 matmul weights on TensorE, 8 pair-stacked
    matmuls per step ([128,64] x [128,2] staggered), f32 PSUM.
  * gold: indirect element-gather of the fp8 arena for all valid (b,t),
    reduced to [128,1] on DVE; host sums partitions and cores.
"""

import math

import numpy as np
import ml_dtypes

import concourse.bacc as bacc
import concourse.bass as bass
import concourse.tile as tile
from concourse import mybir
from concourse.bass_utils import run_bass_kernel_spmd

F32 = mybir.dt.float32
BF16 = mybir.dt.bfloat16
F8 = mybir.dt.float8e4
I32 = mybir.dt.int32
NP_F8 = ml_dtypes.float8_e4m3

B = 128
T = 256
K = 64
START = 62
END = 63
NCORES = 8
BL = B // NCORES          # 16 local batch rows per core
NPAIR = BL // 2           # 8
W = 16                    # timesteps (slots) per arena window
NBLK = T // W             # 16 window blocks in the scan
G = BL * T // 128         # gold gather indices per partition (32)
C = 128.0
C_SCALE = 1.0 / C         # per-step normalizer
LOG_C = math.log(C)
S_STAR = 5.0              # pad-block diagonal (fp8-exact); r = exp(S_STAR)/C
PAD_OFF = -240.0          # pad-block off-diagonal; exp() underflows to 0
SENTINEL = 0x7FFFFF00     # gold index for invalid (padded) positions


def _build_nc(nw):
    """Build the SPMD kernel for an arena of `nw` windows."""
    nc = bacc.Bacc("TRN2", target_bir_lowering=False)

    arena = nc.dram_tensor("arena", [nw, K, W * K], F8, kind="ExternalInput")
    widx = nc.dram_tensor("widx", [128, NPAIR * NBLK], I32, kind="ExternalInput")
    gidx = nc.dram_tensor("gidx", [128, G], I32, kind="ExternalInput")
    init_sc = nc.dram_tensor("init_sc", [K, BL], F32, kind="ExternalInput")
    afin = nc.dram_tensor("afin", [128, BL], F32, kind="ExternalOutput")
    goldp = nc.dram_tensor("goldp", [128, 1], F32, kind="ExternalOutput")

    with tile.TileContext(nc) as tc:
        with (
            tc.tile_pool(name="strips", bufs=2) as strips,
            tc.tile_pool(name="persist", bufs=1) as persist,
            tc.tile_pool(name="pers_psum", bufs=1, space="PSUM") as pers_psum,
        ):
            # ---- persistent tiles -------------------------------------
            rhs_bufs = [
                persist.tile([128, BL], BF16, tag=f"rhs{i}", name=f"rhs{i}")
                for i in range(3)
            ]
            psum_bufs = [
                pers_psum.tile([K, BL], F32, tag=f"ps{i}", name=f"ps{i}")
                for i in range(2)
            ]
            # per-partition window-gather indices: widx_t[p, j*NBLK+blk]
            # = win(row, blk)*K + kf  with row = 2j + (p >= 64), kf = p % 64,
            # so each strip gather is a plain embedding-row gather.
            widx_t = persist.tile(
                [128, NPAIR * NBLK], I32, tag="widx", name="widx_t"
            )
            nc.sync.dma_start(out=widx_t[:], in_=widx[:])
            arena_rows = arena[:].rearrange("n p f -> (n p) f")

            # ---- gold gather (runs concurrently with the scan) --------
            # invalid (padded) positions carry a huge sentinel index; the
            # bounds check silently skips them, leaving the pre-zeroed
            # elements untouched.
            gidx_t = persist.tile([128, G], I32, tag="gidx", name="gidx_t")
            gath = persist.tile([128, G], F8, tag="gath", name="gath")
            gathf = persist.tile([128, G], F32, tag="gathf", name="gathf")
            goldsb = persist.tile([128, 1], F32, tag="goldsb", name="goldsb")
            nc.sync.dma_start(out=gidx_t[:], in_=gidx[:])
            nc.vector.memset(gath[:], 0.0)
            arena_flat = arena[:].rearrange("n p (f one) -> (n p f) one", one=1)
            nc.gpsimd.indirect_dma_start(
                out=gath[:],
                out_offset=None,
                in_=arena_flat,
                in_offset=bass.IndirectOffsetOnAxis(ap=gidx_t[:], axis=0),
                bounds_check=nw * K * W * K - 1,
                oob_is_err=False,
            )
            nc.vector.tensor_copy(gathf[:], gath[:])
            nc.vector.tensor_reduce(
                goldsb[:],
                gathf[:],
                axis=mybir.AxisListType.X,
                op=mybir.AluOpType.add,
            )
            nc.sync.dma_start(out=goldp[:], in_=goldsb[:])

            # ---- init: a_0 = exp(scores[b, 0, START, :]) --------------
            staging = persist.tile([K, BL], F32, tag="staging", name="staging")
            nc.sync.dma_start(out=staging[:], in_=init_sc[:])
            nc.scalar.activation(
                staging[:], staging[:], mybir.ActivationFunctionType.Exp
            )
            rhs0 = rhs_bufs[0]
            nc.vector.memset(rhs0[:], 0.0)
            nc.vector.memset(rhs_bufs[1][:], 0.0)
            nc.vector.memset(rhs_bufs[2][:], 0.0)
            nc.vector.tensor_copy(rhs0[0:64, 0:BL:2], staging[:, 0:BL:2])
            nc.vector.tensor_copy(rhs0[64:128, 1:BL:2], staging[:, 1:BL:2])

            # ---- main scan --------------------------------------------
            rhs_prev = rhs0
            for blk in range(NBLK):
                cur = []
                for j in range(NPAIR):
                    s8 = strips.tile([128, W * K], F8, tag=f"s8_{j}")
                    sE = strips.tile([128, W * K], BF16, tag=f"sE_{j}")
                    nc.vector.memset(s8[:], 0.0)
                    col = j * NBLK + blk
                    nc.gpsimd.indirect_dma_start(
                        out=s8[:],
                        out_offset=None,
                        in_=arena_rows,
                        in_offset=bass.IndirectOffsetOnAxis(
                            ap=widx_t[:, col : col + 1], axis=0
                        ),
                    )
                    nc.scalar.activation(
                        sE[:], s8[:], mybir.ActivationFunctionType.Exp
                    )
                    cur.append(sE)

                for tl in range(W):
                    t = blk * W + tl
                    if t == 0:
                        continue
                    ps = psum_bufs[t % 2]
                    rhs_new = rhs_bufs[t % 3]
                    for j in range(NPAIR):
                        nc.tensor.matmul(
                            out=ps[:, 2 * j : 2 * j + 2],
                            lhsT=cur[j][:, tl * K : (tl + 1) * K],
                            rhs=rhs_prev[:, 2 * j : 2 * j + 2],
                            start=True,
                            stop=True,
                        )
                    # staggered copy psum -> next rhs, with 1/C scaling
                    nc.vector.tensor_scalar_mul(
                        rhs_new[0:64, 0:BL:2], ps[0:64, 0:BL:2], C_SCALE
                    )
                    nc.vector.tensor_scalar_mul(
                        rhs_new[64:128, 1:BL:2], ps[0:64, 1:BL:2], C_SCALE
                    )
                    rhs_prev = rhs_new

            # ---- write back the single final state --------------------
            fstag = persist.tile([128, BL], F32, tag="fstag", name="fstag")
            nc.vector.tensor_copy(fstag[:], rhs_prev[:])
            nc.sync.dma_start(out=afin[:], in_=fstag[:])

            # keep early-dying persist tiles alive to the end of the
            # program so the allocator never recycles their SBUF into the
            # strip buffers (the scheduler and race detector disagree
            # about the indirect-DMA write extent otherwise).
            for t_ in (staging, gath, gathf, gidx_t, goldsb):
                nc.vector.tensor_copy(t_[:], t_[:])

    return nc


_NC_CACHE = {}


def _get_nc(nw):
    if nw not in _NC_CACHE:
        nc = _build_nc(nw)
        nc.finalize()
        _NC_CACHE[nw] = nc
    return _NC_CACHE[nw]


def _assign_rows(lengths):
    """LPT-balance rows over cores on window count; exactly BL rows/core."""
    nwin = [(int(l) + W - 1) // W for l in lengths]
    order = np.argsort(-np.asarray(nwin), kind="stable")
    loads = [0] * NCORES
    counts = [0] * NCORES
    assign = [[] for _ in range(NCORES)]
    for r in order:
        cands = [c for c in range(NCORES) if counts[c] < BL]
        c = min(cands, key=lambda c: loads[c])
        assign[c].append(int(r))
        loads[c] += nwin[r]
        counts[c] += 1
    return assign, loads


def _pad_slot_f8():
    """One [K, K] pad block: diag S_STAR, off-diag PAD_OFF, as fp8."""
    blk = np.full((K, K), PAD_OFF, np.float32)
    np.fill_diagonal(blk, S_STAR)
    return blk.astype(NP_F8)


def _make_in_maps(scores, targets, lengths):
    scores = np.asarray(scores, dtype=np.float32)
    targets = np.asarray(targets).astype(np.int64)
    lengths = np.asarray(lengths).astype(np.int64)

    assign, loads = _assign_rows(lengths)
    nw = max(loads) + 1  # + shared identity window 0
    nw = ((nw + 3) // 4) * 4  # bucket for compile-cache stability

    pad8 = _pad_slot_f8()                      # [K, K] fp8
    idw = np.broadcast_to(pad8[:, None, :], (K, W, K)).reshape(K, W * K)

    in_maps = []
    row_map = []  # (core, slot) per global row, parallel to assign order
    kf_col = np.arange(128, dtype=np.int32) % K   # [128]
    for c in range(NCORES):
        rows = assign[c]
        arena = np.zeros((nw, K, W * K), NP_F8)
        arena[0] = idw
        win_of = np.zeros((BL, NBLK), np.int32)    # window id per (slot, blk)
        init_sc = np.empty((K, BL), np.float32)
        gflat = np.full((BL, T), SENTINEL, np.int64)

        start = 1
        for slot, r in enumerate(rows):
            L = int(lengths[r])
            nwj = (L + W - 1) // W
            q = scores[r, :L].astype(NP_F8)            # [L, K, K]
            if nwj * W > L:
                padn = nwj * W - L
                q = np.concatenate(
                    [q, np.broadcast_to(pad8, (padn, K, K))], axis=0
                )
            arena[start : start + nwj] = (
                q.reshape(nwj, W, K, K).transpose(0, 2, 1, 3).reshape(nwj, K, W * K)
            )
            win_of[slot, :nwj] = start + np.arange(nwj)
            init_sc[:, slot] = scores[r, 0, START, :]

            # gold element indices into the arena (fp8 elements)
            t_idx = np.arange(L)
            tg = targets[r, :L]
            kf, kto = tg // K, tg % K
            win = start + t_idx // W
            gflat[slot, :L] = (
                (win * K + kf) * (W * K) + (t_idx % W) * K + kto
            )
            start += nwj

        # widx[p, j*NBLK+blk] = win(2j + (p>=64), blk)*K + p%64
        widx = np.empty((128, NPAIR * NBLK), np.int32)
        for j in range(NPAIR):
            cols = slice(j * NBLK, (j + 1) * NBLK)
            widx[0:64, cols] = (
                win_of[2 * j][None, :] * K + kf_col[0:64, None]
            )
            widx[64:128, cols] = (
                win_of[2 * j + 1][None, :] * K + kf_col[64:128, None]
            )

        in_maps.append(
            {
                "arena": arena,
                "widx": widx,
                "gidx": np.ascontiguousarray(
                    gflat.reshape(128, G).astype(np.int32)
                ),
                "init_sc": np.ascontiguousarray(init_sc),
            }
        )
        row_map.append(rows)
    return in_maps, row_map, lengths, nw


def _combine(results, row_map, lengths):
    r_corr = math.exp(S_STAR) / C
    log_r = math.log(r_corr)
    all_scores = 0.0
    gold_total = 0.0
    for c in range(NCORES):
        gold_total += float(np.sum(results[c]["goldp"], dtype=np.float64))
        af = results[c]["afin"]  # [128, BL]
        for slot, r in enumerate(row_map[c]):
            tau = int(lengths[r]) - 1
            a_end = float(af[(slot % 2) * 64 + END, slot])
            all_scores += (
                math.log(a_end) + tau * LOG_C - (T - 1 - tau) * log_r
            )
    return np.float32((all_scores - gold_total) / B)


def kernel(scores, targets, lengths, trace=False):
    in_maps, row_map, ln, nw = _make_in_maps(scores, targets, lengths)
    nc = _get_nc(nw)
    res = run_bass_kernel_spmd(
        nc, in_maps, core_ids=list(range(NCORES)), trace=trace
    )
    out = _combine(res.results, row_map, ln)
    if trace:
        return out, res
    return out
